# revision 11
# baseline (speedup 1.0000x reference)
"""Trainium2 Bass kernel for nn_MemoryGame (scatter_memory).

Math (see reference):
    P = 8192, T = 4 timesteps, N_ITER = 50 attractor iterations.
    per t: h0 = f_p(tile(g_t, 128));  50x: h = f_p(kappa*h + h*(h@M))
           p = outer(x_t, g_t).ravel()
           loss_t = sum|p - h|
           M = lamda*M + yita*outer(p+h, p-h)
    output = sum_t loss_t   (scalar, fp32)

Distribution: M column-sharded over 8 cores (core k owns columns
[k*1024,(k+1)*1024)).  Each core computes its 1024-slice of a = h@M
exactly (full contraction done locally), applies the pointwise update to
its slice, and an AllGather rebuilds the full h on every core each
iteration.

Numerics: the attractor is chaotic.  Measured on this problem, matmul
operands need ~15+ mantissa bits to keep the final loss within ~1e-4;
fp16/bf16/float32r (11 bits) all land at 1e-2..2e-2.  So everything is
kept in exact fp32.  Plain fp32 PE matmul runs at 4 cycles/row, so the
GEMV is split: the PE handles PE_CHUNKS contraction chunks, and
ScalarE (per-partition-scale multiply) + VectorE (accumulate) handle
the rest into a 2-D accumulator which the PE then partition-reduces
via a ones-vector matmul accumulated into the same PSUM bank.

Memory: a 32MB fp32 shard exceeds the ~26MB usable SBUF, so RESIDENT
chunks stay SBUF-resident and the rest stream from HBM every
iteration, hidden behind the compute.

The lamda*M decay is folded into a scalar: we keep Mt = M0 +
sum_s (yita/lamda^(s+1)) u_s v_s^T and apply a = lamda^t * (h @ Mt),
saving a full read-modify-write scale pass over M per timestep.

Layout: contraction index i = p*64 + c (p = partition, c = chunk),
i.e. M's rows viewed as M0.reshape(128, 64, P); the post-AllGather
full h then loads straight into SBUF [128, 64] with no transpose, and
chunk c's stationary operand is h_sb[:, c].
"""

import os
import numpy as np

N_CORES = 8
P_DIM = 8192
NXD, NGD = 128, 64
T_STEPS = 4
N_ITER = 50
KAPPA, LAMDA, YITA = 0.8, 0.9, 0.1
NEG = 0.01

NCHUNK = 64                 # contraction chunks (128 rows each)
J_LOC = P_DIM // N_CORES    # 1024 columns per core
RESIDENT = 38               # chunks kept in SBUF
STREAMED = NCHUNK - RESIDENT
STREAM_BUFS = 4
PE_CHUNKS = 29              # chunks the PE computes; rest go to ACT+DVE

_cache = {}


def _f_p(v):
    c = np.clip(v, -1.0, 1.0)
    return np.where(c >= 0, c, NEG * c).astype(np.float32)


def build_program(debug_h=False, n_iter=None, t_run=None):
    import concourse.bacc as bacc
    import concourse.mybir as mybir
    import concourse.tile as tile

    if n_iter is None:
        n_iter = N_ITER
    if t_run is None:
        t_run = T_STEPS

    f32 = mybir.dt.float32
    AF = mybir.ActivationFunctionType
    ALU = mybir.AluOpType

    nc = bacc.Bacc(None, target_bir_lowering=False, num_devices=N_CORES)

    # register KAPPA so activation(bias=KAPPA) finds a const AP
    kap = nc.alloc_sbuf_tensor("const-kappa", [128, 1], f32)
    nc.gpsimd.memset(kap.ap(), float(KAPPA))
    nc.const_aps.aps[(f32, float(KAPPA))] = kap.ap()
    nc.all_engine_barrier()

    # ---- I/O ----
    m_res_in = nc.dram_tensor("m_res_in", [128, RESIDENT * J_LOC], f32, kind="ExternalInput")
    m_strm_in = nc.dram_tensor("m_strm_in", [STREAMED, 128, J_LOC], f32, kind="ExternalInput")
    h0_sb_in = nc.dram_tensor("h0_sb_in", [T_STEPS, 128, NGD], f32, kind="ExternalInput")
    h0_loc_in = nc.dram_tensor("h0_loc_in", [T_STEPS, 1, J_LOC], f32, kind="ExternalInput")
    p_sb_in = nc.dram_tensor("p_sb_in", [T_STEPS, 128, NGD], f32, kind="ExternalInput")
    p_loc_in = nc.dram_tensor("p_loc_in", [T_STEPS, 1, J_LOC], f32, kind="ExternalInput")
    loss_out = nc.dram_tensor("loss_out", [1, 1], f32, kind="ExternalOutput")
    if debug_h:
        hdbg_out = nc.dram_tensor("hdbg_out", [n_iter, 1, J_LOC], f32, kind="ExternalOutput")

    # chunk engine assignment: PE gets resident chunks [0, PE_CHUNKS);
    # ACT+DVE get the rest (remaining resident + all streamed).
    dve_chunks = list(range(PE_CHUNKS, NCHUNK))

    with tile.TileContext(nc) as tc:
        with (
            tc.tile_pool(name="strm_pool", bufs=STREAM_BUFS) as strm_pool,
            tc.tile_pool(name="tmp_pool", bufs=3) as tmp_pool,
            tc.tile_pool(name="pw_pool", bufs=3) as pw_pool,
            tc.tile_pool(name="state_pool", bufs=1) as state_pool,
            tc.tile_pool(name="psum_pool", bufs=2, space="PSUM") as psum_pool,
            tc.tile_pool(name="vb_psum_pool", bufs=2, space="PSUM") as vb_psum_pool,
            tc.tile_pool(name="dram_pool", bufs=1, space="DRAM") as dram_pool,
        ):
            # ---- persistent SBUF state ----
            m_res = state_pool.tile([128, RESIDENT * J_LOC], f32)
            acc2d = state_pool.tile([128, J_LOC], f32)    # DVE-side accumulator
            h_sb = state_pool.tile([128, NGD], f32)       # full h, [p, c] = h[p*64+c]
            h_row = state_pool.tile([1, J_LOC], f32)      # local slice of h
            p_sb = state_pool.tile([128, NGD], f32)
            u_eta = state_pool.tile([128, NGD], f32)
            v_bcast = state_pool.tile([128, J_LOC], f32)
            ones_row = state_pool.tile([1, 128], f32)     # K=1 broadcast trick
            ones_col = state_pool.tile([128, 1], f32)     # partition reduction
            loss_acc = state_pool.tile([1, 1], f32)
            loss_tmp = state_pool.tile([1, 1], f32)

            # streamed part of M lives in (internal) DRAM so we can update it
            m_strm = dram_pool.tile([STREAMED, 128, J_LOC], f32)
            cc_in = dram_pool.tile([1, J_LOC], f32)

            # ---- init ----
            nc.gpsimd.memset(ones_row[:], 1.0)
            nc.gpsimd.memset(ones_col[:], 1.0)
            nc.gpsimd.memset(loss_acc[:], 0.0)
            n_ld = 8
            step = (RESIDENT * J_LOC) // n_ld
            for i in range(n_ld):
                nc.sync.dma_start(m_res[:, i * step:(i + 1) * step],
                                  m_res_in[:, i * step:(i + 1) * step])
            for s in range(STREAMED):
                nc.sync.dma_start(m_strm[s], m_strm_in[s])

            def m_chunk(c, t, it):
                """AP for chunk c of M (SBUF-resident or freshly streamed)."""
                if c < RESIDENT:
                    return m_res[:, c * J_LOC:(c + 1) * J_LOC]
                tile_c = strm_pool.tile([128, J_LOC], f32, tag="mstrm",
                                        name=f"chk_{t}_{it}_{c}")
                nc.sync.dma_start(tile_c[:], m_strm[c - RESIDENT])
                return tile_c[:]

            for t in range(t_run):
                nc.sync.dma_start(h_sb[:], h0_sb_in[t])
                nc.sync.dma_start(h_row[:], h0_loc_in[t])
                nc.sync.dma_start(p_sb[:], p_sb_in[t])
                scale_t = float(LAMDA ** t)

                for it in range(n_iter):
                    with nc.named_scope(f"iter_t{t}_i{it}"):
                        acc = psum_pool.tile([1, J_LOC], f32, tag="acc",
                                             name=f"acc_{t}_{it}")

                        # --- DVE/ACT side: acc2d[p,j] = sum_c M[p,c,j]*h_sb[p,c]
                        for n, c in enumerate(dve_chunks):
                            mc = m_chunk(c, t, it)
                            hcol = h_sb[:, c:c + 1]
                            if n == 0:
                                nc.vector.tensor_scalar_mul(acc2d[:], mc, hcol)
                            else:
                                tmp = tmp_pool.tile([128, J_LOC], f32, tag="tmp",
                                                    name=f"tmp_{t}_{it}_{c}")
                                nc.scalar.activation(tmp[:], mc, AF.Copy, scale=hcol)
                                nc.vector.tensor_tensor(acc2d[:], acc2d[:], tmp[:], ALU.add)

                        # --- PE side: psum chunks, then += ones^T @ acc2d
                        for n in range(PE_CHUNKS):
                            lhsT = h_sb[:, n:n + 1]
                            rhs = m_res[:, n * J_LOC:(n + 1) * J_LOC]
                            nc.tensor.matmul(acc[:, 0:512], lhsT, rhs[:, 0:512],
                                             start=(n == 0), stop=False,
                                             skip_group_check=True)
                            nc.tensor.matmul(acc[:, 512:1024], lhsT, rhs[:, 512:1024],
                                             start=(n == 0), stop=False,
                                             skip_group_check=True)
                        nc.tensor.matmul(acc[:, 0:512], ones_col[:], acc2d[:, 0:512],
                                         start=False, stop=True, skip_group_check=True)
                        nc.tensor.matmul(acc[:, 512:1024], ones_col[:], acc2d[:, 512:1024],
                                         start=False, stop=True, skip_group_check=True)

                        # --- pointwise: h = f_p(kappa*h + h*a) with a = lamda^t*acc
                        s_t = pw_pool.tile([1, J_LOC], f32, tag="pw", name=f"s_{t}_{it}")
                        nc.scalar.activation(s_t[:], acc[:], AF.Identity,
                                             bias=float(KAPPA), scale=scale_t)
                        w_t = pw_pool.tile([1, J_LOC], f32, tag="pw", name=f"w_{t}_{it}")
                        nc.vector.tensor_tensor(w_t[:], h_row[:], s_t[:], ALU.mult)
                        c2 = pw_pool.tile([1, J_LOC], f32, tag="pw", name=f"c2_{t}_{it}")
                        nc.vector.tensor_scalar(c2[:], w_t[:], 1.0, -1.0, ALU.min, ALU.max)
                        nc.scalar.activation(h_row[:], c2[:], AF.Lrelu, alpha=float(NEG))

                        # --- exchange h slices (each AllGather needs a fresh
                        # Shared output buffer: single-writer constraint)
                        cc_out = dram_pool.tile([1, P_DIM], f32, addr_space="Shared",
                                                name=f"cc_out_{t}_{it}",
                                                tag=f"cc_out_{t}_{it}")
                        nc.sync.dma_start(cc_in[:], h_row[:])
                        nc.gpsimd.collective_compute(
                            "AllGather", ALU.bypass,
                            replica_groups=[list(range(N_CORES))],
                            ins=[cc_in[:].opt()],
                            outs=[cc_out[:].opt()],
                        )
                        cc_v = cc_out[:].rearrange("o (p c) -> (o p) c", c=NGD)
                        nc.sync.dma_start(h_sb[0:64, :], cc_v[0:64, :])
                        nc.sync.dma_start(h_sb[64:128, :], cc_v[64:128, :])
                        if debug_h:
                            nc.sync.dma_start(hdbg_out[it], h_row[:])

                # ---- timestep tail: loss, then Hebbian update if t < 3 ----
                p_loc = pw_pool.tile([1, J_LOC], f32, tag="pw", name=f"ploc_{t}")
                nc.sync.dma_start(p_loc[:], p_loc_in[t])
                v_row = pw_pool.tile([1, J_LOC], f32, tag="pw", name=f"vrow_{t}")
                nc.vector.tensor_tensor(v_row[:], p_loc[:], h_row[:], ALU.subtract)
                nc.vector.tensor_reduce(loss_tmp[:], v_row[:],
                                        mybir.AxisListType.X, ALU.add,
                                        apply_absolute_value=True)
                nc.vector.tensor_tensor(loss_acc[:], loss_acc[:], loss_tmp[:], ALU.add)

                if t < t_run - 1:
                    coef = float(YITA / (LAMDA ** (t + 1)))
                    nc.vector.tensor_tensor(u_eta[:], p_sb[:], h_sb[:], ALU.add)
                    nc.vector.tensor_scalar_mul(u_eta[:], u_eta[:], coef)
                    # v_bcast[p, j] = v_row[j] via K=1 ones matmul
                    for half in range(2):
                        vb_ps = vb_psum_pool.tile([128, 512], f32, tag="vb",
                                                  name=f"vb_{t}_{half}")
                        nc.tensor.matmul(vb_ps[:], ones_row[:],
                                         v_row[:, half * 512:(half + 1) * 512],
                                         start=True, stop=True)
                        nc.vector.tensor_copy(v_bcast[:, half * 512:(half + 1) * 512],
                                              vb_ps[:])
                    # M[:, c, :] += u_eta[:, c] * v_bcast
                    for c in range(NCHUNK):
                        tmp = tmp_pool.tile([128, J_LOC], f32, tag="tmp",
                                            name=f"upd_tmp_{t}_{c}")
                        nc.scalar.activation(tmp[:], v_bcast[:], AF.Copy,
                                             scale=u_eta[:, c:c + 1])
                        if c < RESIDENT:
                            mc = m_res[:, c * J_LOC:(c + 1) * J_LOC]
                            nc.vector.tensor_tensor(mc, mc, tmp[:], ALU.add)
                        else:
                            buf = strm_pool.tile([128, J_LOC], f32, tag="mstrm",
                                                 name=f"upd_buf_{t}_{c}")
                            nc.sync.dma_start(buf[:], m_strm[c - RESIDENT])
                            nc.vector.tensor_tensor(buf[:], buf[:], tmp[:], ALU.add)
                            nc.sync.dma_start(m_strm[c - RESIDENT], buf[:])

            nc.sync.dma_start(loss_out[:], loss_acc[:])

    nc.compile()
    return nc


def prepare_inputs(x, g, M0):
    """Host-side sharding/layout prep. Returns list of per-core input maps."""
    x = np.asarray(x, dtype=np.float32)
    g = np.asarray(g, dtype=np.float32)
    M0 = np.ascontiguousarray(np.asarray(M0, dtype=np.float32))

    # i = p*64 + c  =>  view rows as [128, 64]
    M_view = M0.reshape(128, NCHUNK, P_DIM)

    h0_all = np.zeros((T_STEPS, 128, NGD), np.float32)
    p_all = np.zeros((T_STEPS, 128, NGD), np.float32)
    h0_flat = np.zeros((T_STEPS, P_DIM), np.float32)
    p_flat = np.zeros((T_STEPS, P_DIM), np.float32)
    for t in range(T_STEPS):
        q = np.tile(g[t], NXD)                    # g @ W_tile
        h0 = _f_p(q)
        p = np.outer(x[t], g[t]).reshape(P_DIM).astype(np.float32)
        h0_flat[t] = h0
        p_flat[t] = p
        h0_all[t] = h0.reshape(128, NGD)
        p_all[t] = p.reshape(128, NGD)

    in_maps = []
    for k in range(N_CORES):
        shard = M_view[:, :, k * J_LOC:(k + 1) * J_LOC]      # [128, 64, 1024]
        m_res = np.ascontiguousarray(
            shard[:, :RESIDENT, :]).reshape(128, RESIDENT * J_LOC)
        m_strm = np.ascontiguousarray(
            shard[:, RESIDENT:, :].transpose(1, 0, 2))        # [STREAMED, 128, 1024]
        in_maps.append({
            "m_res_in": m_res,
            "m_strm_in": m_strm,
            "h0_sb_in": h0_all,
            "h0_loc_in": h0_flat[:, k * J_LOC:(k + 1) * J_LOC].reshape(T_STEPS, 1, J_LOC).copy(),
            "p_sb_in": p_all,
            "p_loc_in": p_flat[:, k * J_LOC:(k + 1) * J_LOC].reshape(T_STEPS, 1, J_LOC).copy(),
        })
    return in_maps


def kernel(x, g, M0):
    from concourse.bass_utils import run_bass_kernel_spmd

    in_maps = prepare_inputs(x, g, M0)
    if "nc" not in _cache:
        _cache["nc"] = build_program()
    nc = _cache["nc"]
    trace = bool(int(os.environ.get("MG_TRACE", "0")))
    res = run_bass_kernel_spmd(nc, in_maps, core_ids=list(range(N_CORES)),
                               trace=trace)
    _cache["last_result"] = res
    total = np.float32(0.0)
    for k in range(N_CORES):
        total += res.results[k]["loss_out"][0, 0]
    return np.float32(total)


# revision 12
# speedup vs baseline: 1.0842x; 1.0842x over previous
"""Trainium2 Bass kernel for nn_MemoryGame (scatter_memory).

Math (see reference):
    P = 8192, T = 4 timesteps, N_ITER = 50 attractor iterations.
    per t: h0 = f_p(tile(g_t, 128));  50x: h = f_p(kappa*h + h*(h@M))
           p = outer(x_t, g_t).ravel()
           loss_t = sum|p - h|
           M = lamda*M + yita*outer(p+h, p-h)
    output = sum_t loss_t   (scalar, fp32)

Distribution: M column-sharded over 8 cores (core k owns columns
[k*1024,(k+1)*1024)).  Each core computes its 1024-slice of a = h@M
exactly (full contraction done locally), applies the pointwise update to
its slice, and an AllGather rebuilds the full h on every core each
iteration.

Numerics: the attractor is chaotic.  Measured on this problem, matmul
operands need ~15+ mantissa bits to keep the final loss within ~1e-4;
fp16/bf16/float32r (11 bits) all land at 1e-2..2e-2.  So everything is
kept in exact fp32.  Plain fp32 PE matmul runs at 4 cycles/row, so the
GEMV is split: the PE handles PE_CHUNKS contraction chunks, and
ScalarE (per-partition-scale multiply) + VectorE (accumulate) handle
the rest into a 2-D accumulator which the PE then partition-reduces
via a ones-vector matmul accumulated into the same PSUM bank.

Memory: a 32MB fp32 shard exceeds the ~26MB usable SBUF, so RESIDENT
chunks stay SBUF-resident and the rest stream from HBM every
iteration, hidden behind the compute.

The lamda*M decay is folded into a scalar: we keep Mt = M0 +
sum_s (yita/lamda^(s+1)) u_s v_s^T and apply a = lamda^t * (h @ Mt),
saving a full read-modify-write scale pass over M per timestep.

Layout: contraction index i = p*64 + c (p = partition, c = chunk),
i.e. M's rows viewed as M0.reshape(128, 64, P); the post-AllGather
full h then loads straight into SBUF [128, 64] with no transpose, and
chunk c's stationary operand is h_sb[:, c].
"""

import os
import numpy as np

N_CORES = 8
P_DIM = 8192
NXD, NGD = 128, 64
T_STEPS = 4
N_ITER = 50
KAPPA, LAMDA, YITA = 0.8, 0.9, 0.1
NEG = 0.01

NCHUNK = 64                 # contraction chunks (128 rows each)
J_LOC = P_DIM // N_CORES    # 1024 columns per core
RESIDENT = 38               # chunks kept in SBUF
STREAMED = NCHUNK - RESIDENT
STREAM_BUFS = 4
PE_CHUNKS = 31              # chunks the PE computes (as bf16 hi/lo pairs);
                            # the rest (fp32) go to ACT+DVE

_cache = {}


def _f_p(v):
    c = np.clip(v, -1.0, 1.0)
    return np.where(c >= 0, c, NEG * c).astype(np.float32)


def build_program(debug_h=False, n_iter=None, t_run=None):
    import concourse.bacc as bacc
    import concourse.mybir as mybir
    import concourse.tile as tile

    if n_iter is None:
        n_iter = N_ITER
    if t_run is None:
        t_run = T_STEPS

    f32 = mybir.dt.float32
    AF = mybir.ActivationFunctionType
    ALU = mybir.AluOpType

    nc = bacc.Bacc(None, target_bir_lowering=False, num_devices=N_CORES)

    # register KAPPA so activation(bias=KAPPA) finds a const AP
    kap = nc.alloc_sbuf_tensor("const-kappa", [128, 1], f32)
    nc.gpsimd.memset(kap.ap(), float(KAPPA))
    nc.const_aps.aps[(f32, float(KAPPA))] = kap.ap()
    nc.all_engine_barrier()

    # ---- I/O ----
    bf16 = mybir.dt.bfloat16
    m_hi_in = nc.dram_tensor("m_hi_in", [128, PE_CHUNKS * J_LOC], bf16, kind="ExternalInput")
    m_lo_in = nc.dram_tensor("m_lo_in", [128, PE_CHUNKS * J_LOC], bf16, kind="ExternalInput")
    m_res_in = nc.dram_tensor("m_res_in", [128, (RESIDENT - PE_CHUNKS) * J_LOC], f32, kind="ExternalInput")
    m_strm_in = nc.dram_tensor("m_strm_in", [STREAMED, 128, J_LOC], f32, kind="ExternalInput")
    h0_sb_in = nc.dram_tensor("h0_sb_in", [T_STEPS, 128, NGD], f32, kind="ExternalInput")
    h0_loc_in = nc.dram_tensor("h0_loc_in", [T_STEPS, 1, J_LOC], f32, kind="ExternalInput")
    p_sb_in = nc.dram_tensor("p_sb_in", [T_STEPS, 128, NGD], f32, kind="ExternalInput")
    p_loc_in = nc.dram_tensor("p_loc_in", [T_STEPS, 1, J_LOC], f32, kind="ExternalInput")
    loss_out = nc.dram_tensor("loss_out", [1, 1], f32, kind="ExternalOutput")
    if debug_h:
        hdbg_out = nc.dram_tensor("hdbg_out", [n_iter, 1, J_LOC], f32, kind="ExternalOutput")

    # chunk engine assignment: PE gets resident chunks [0, PE_CHUNKS);
    # ACT+DVE get the rest (remaining resident + all streamed).
    dve_chunks = list(range(PE_CHUNKS, NCHUNK))

    with tile.TileContext(nc) as tc:
        with (
            tc.tile_pool(name="strm_pool", bufs=STREAM_BUFS) as strm_pool,
            tc.tile_pool(name="tmp_pool", bufs=3) as tmp_pool,
            tc.tile_pool(name="pw_pool", bufs=3) as pw_pool,
            tc.tile_pool(name="state_pool", bufs=1) as state_pool,
            tc.tile_pool(name="psum_pool", bufs=2, space="PSUM") as psum_pool,
            tc.tile_pool(name="vb_psum_pool", bufs=2, space="PSUM") as vb_psum_pool,
            tc.tile_pool(name="dram_pool", bufs=1, space="DRAM") as dram_pool,
        ):
            # ---- persistent SBUF state ----
            m_hi = state_pool.tile([128, PE_CHUNKS * J_LOC], bf16)
            m_lo = state_pool.tile([128, PE_CHUNKS * J_LOC], bf16)
            m_res = state_pool.tile([128, (RESIDENT - PE_CHUNKS) * J_LOC], f32)
            h_hi = state_pool.tile([128, NGD], bf16)
            h_lo = state_pool.tile([128, NGD], bf16)
            acc2d = state_pool.tile([128, J_LOC], f32)    # DVE-side accumulator
            h_sb = state_pool.tile([128, NGD], f32)       # full h, [p, c] = h[p*64+c]
            h_row = state_pool.tile([1, J_LOC], f32)      # local slice of h
            p_sb = state_pool.tile([128, NGD], f32)
            u_eta = state_pool.tile([128, NGD], f32)
            v_bcast = state_pool.tile([128, J_LOC], f32)
            ones_row = state_pool.tile([1, 128], f32)     # K=1 broadcast trick
            ones_col = state_pool.tile([128, 1], f32)     # partition reduction
            loss_acc = state_pool.tile([1, 1], f32)
            loss_tmp = state_pool.tile([1, 1], f32)

            # streamed part of M lives in (internal) DRAM so we can update it
            m_strm = dram_pool.tile([STREAMED, 128, J_LOC], f32)
            cc_in = dram_pool.tile([1, J_LOC], f32)

            # ---- init ----
            nc.gpsimd.memset(ones_row[:], 1.0)
            nc.gpsimd.memset(ones_col[:], 1.0)
            nc.gpsimd.memset(loss_acc[:], 0.0)
            n_ld = 8
            for big, big_in in ((m_hi, m_hi_in), (m_lo, m_lo_in)):
                step = (PE_CHUNKS * J_LOC) // n_ld
                rem = (PE_CHUNKS * J_LOC) - n_ld * step
                for i in range(n_ld):
                    e = (i + 1) * step + (rem if i == n_ld - 1 else 0)
                    nc.sync.dma_start(big[:, i * step:e], big_in[:, i * step:e])
            for i in range(RESIDENT - PE_CHUNKS):
                nc.sync.dma_start(m_res[:, i * J_LOC:(i + 1) * J_LOC],
                                  m_res_in[:, i * J_LOC:(i + 1) * J_LOC])
            for s in range(STREAMED):
                nc.sync.dma_start(m_strm[s], m_strm_in[s])

            def m_chunk(c, t, it):
                """fp32 AP for DVE-side chunk c (resident or freshly streamed)."""
                if c < RESIDENT:
                    i = c - PE_CHUNKS
                    return m_res[:, i * J_LOC:(i + 1) * J_LOC]
                tile_c = strm_pool.tile([128, J_LOC], f32, tag="mstrm",
                                        name=f"chk_{t}_{it}_{c}")
                nc.sync.dma_start(tile_c[:], m_strm[c - RESIDENT])
                return tile_c[:]

            for t in range(t_run):
                nc.sync.dma_start(h_sb[:], h0_sb_in[t])
                nc.sync.dma_start(h_row[:], h0_loc_in[t])
                nc.sync.dma_start(p_sb[:], p_sb_in[t])
                scale_t = float(LAMDA ** t)

                for it in range(n_iter):
                    with nc.named_scope(f"iter_t{t}_i{it}"):
                        acc = psum_pool.tile([1, J_LOC], f32, tag="acc",
                                             name=f"acc_{t}_{it}")

                        # --- DVE/ACT side: acc2d[p,j] = sum_c M[p,c,j]*h_sb[p,c]
                        for n, c in enumerate(dve_chunks):
                            mc = m_chunk(c, t, it)
                            hcol = h_sb[:, c:c + 1]
                            if n == 0:
                                nc.vector.tensor_scalar_mul(acc2d[:], mc, hcol)
                            else:
                                tmp = tmp_pool.tile([128, J_LOC], f32, tag="tmp",
                                                    name=f"tmp_{t}_{it}_{c}")
                                nc.scalar.activation(tmp[:], mc, AF.Copy, scale=hcol)
                                nc.vector.tensor_tensor(acc2d[:], acc2d[:], tmp[:], ALU.add)

                        # --- split h into bf16 hi/lo for the PE passes
                        nc.scalar.activation(h_hi[:], h_sb[:], AF.Copy)
                        nc.vector.tensor_tensor(h_lo[:], h_sb[:], h_hi[:], ALU.subtract)

                        # --- PE side: bf16 3-pass chunks, then += ones^T @ acc2d
                        for n in range(PE_CHUNKS):
                            sl = slice(n * J_LOC, (n + 1) * J_LOC)
                            first = n == 0
                            for pi, (hh, mm) in enumerate(
                                    ((h_hi, m_hi), (h_hi, m_lo), (h_lo, m_hi))):
                                lhsT = hh[:, n:n + 1]
                                rhs = mm[:, sl]
                                st = first and pi == 0
                                nc.tensor.matmul(acc[:, 0:512], lhsT, rhs[:, 0:512],
                                                 start=st, stop=False,
                                                 skip_group_check=True)
                                nc.tensor.matmul(acc[:, 512:1024], lhsT,
                                                 rhs[:, 512:1024],
                                                 start=st, stop=False,
                                                 skip_group_check=True)
                        nc.tensor.matmul(acc[:, 0:512], ones_col[:], acc2d[:, 0:512],
                                         start=False, stop=True, skip_group_check=True)
                        nc.tensor.matmul(acc[:, 512:1024], ones_col[:], acc2d[:, 512:1024],
                                         start=False, stop=True, skip_group_check=True)

                        # --- pointwise: h = f_p(kappa*h + h*a) with a = lamda^t*acc
                        s_t = pw_pool.tile([1, J_LOC], f32, tag="pw", name=f"s_{t}_{it}")
                        nc.scalar.activation(s_t[:], acc[:], AF.Identity,
                                             bias=float(KAPPA), scale=scale_t)
                        w_t = pw_pool.tile([1, J_LOC], f32, tag="pw", name=f"w_{t}_{it}")
                        nc.vector.tensor_tensor(w_t[:], h_row[:], s_t[:], ALU.mult)
                        c2 = pw_pool.tile([1, J_LOC], f32, tag="pw", name=f"c2_{t}_{it}")
                        nc.vector.tensor_scalar(c2[:], w_t[:], 1.0, -1.0, ALU.min, ALU.max)
                        nc.scalar.activation(h_row[:], c2[:], AF.Lrelu, alpha=float(NEG))

                        # --- exchange h slices (each AllGather needs a fresh
                        # Shared output buffer: single-writer constraint)
                        cc_out = dram_pool.tile([1, P_DIM], f32, addr_space="Shared",
                                                name=f"cc_out_{t}_{it}",
                                                tag=f"cc_out_{t}_{it}")
                        nc.sync.dma_start(cc_in[:], h_row[:])
                        nc.gpsimd.collective_compute(
                            "AllGather", ALU.bypass,
                            replica_groups=[list(range(N_CORES))],
                            ins=[cc_in[:].opt()],
                            outs=[cc_out[:].opt()],
                        )
                        cc_v = cc_out[:].rearrange("o (p c) -> (o p) c", c=NGD)
                        nc.sync.dma_start(h_sb[0:64, :], cc_v[0:64, :])
                        nc.sync.dma_start(h_sb[64:128, :], cc_v[64:128, :])
                        if debug_h:
                            nc.sync.dma_start(hdbg_out[it], h_row[:])

                # ---- timestep tail: loss, then Hebbian update if t < 3 ----
                p_loc = pw_pool.tile([1, J_LOC], f32, tag="pw", name=f"ploc_{t}")
                nc.sync.dma_start(p_loc[:], p_loc_in[t])
                v_row = pw_pool.tile([1, J_LOC], f32, tag="pw", name=f"vrow_{t}")
                nc.vector.tensor_tensor(v_row[:], p_loc[:], h_row[:], ALU.subtract)
                nc.vector.tensor_reduce(loss_tmp[:], v_row[:],
                                        mybir.AxisListType.X, ALU.add,
                                        apply_absolute_value=True)
                nc.vector.tensor_tensor(loss_acc[:], loss_acc[:], loss_tmp[:], ALU.add)

                if t < t_run - 1:
                    coef = float(YITA / (LAMDA ** (t + 1)))
                    nc.vector.tensor_tensor(u_eta[:], p_sb[:], h_sb[:], ALU.add)
                    nc.vector.tensor_scalar_mul(u_eta[:], u_eta[:], coef)
                    # v_bcast[p, j] = v_row[j] via K=1 ones matmul
                    for half in range(2):
                        vb_ps = vb_psum_pool.tile([128, 512], f32, tag="vb",
                                                  name=f"vb_{t}_{half}")
                        nc.tensor.matmul(vb_ps[:], ones_row[:],
                                         v_row[:, half * 512:(half + 1) * 512],
                                         start=True, stop=True)
                        nc.vector.tensor_copy(v_bcast[:, half * 512:(half + 1) * 512],
                                              vb_ps[:])
                    # M[:, c, :] += u_eta[:, c] * v_bcast
                    for c in range(NCHUNK):
                        tmp = tmp_pool.tile([128, J_LOC], f32, tag="tmp",
                                            name=f"upd_tmp_{t}_{c}")
                        nc.scalar.activation(tmp[:], v_bcast[:], AF.Copy,
                                             scale=u_eta[:, c:c + 1])
                        if c < PE_CHUNKS:
                            # X = hi + lo + tmp; re-split to bf16 pair
                            sl = slice(c * J_LOC, (c + 1) * J_LOC)
                            xf = tmp_pool.tile([128, J_LOC], f32, tag="tmp",
                                               name=f"upd_x_{t}_{c}")
                            nc.vector.tensor_tensor(xf[:], m_hi[:, sl], m_lo[:, sl],
                                                    ALU.add)
                            nc.vector.tensor_tensor(xf[:], xf[:], tmp[:], ALU.add)
                            nc.vector.tensor_copy(m_hi[:, sl], xf[:])
                            nc.vector.tensor_tensor(m_lo[:, sl], xf[:], m_hi[:, sl],
                                                    ALU.subtract)
                        elif c < RESIDENT:
                            i = c - PE_CHUNKS
                            mc = m_res[:, i * J_LOC:(i + 1) * J_LOC]
                            nc.vector.tensor_tensor(mc, mc, tmp[:], ALU.add)
                        else:
                            buf = strm_pool.tile([128, J_LOC], f32, tag="mstrm",
                                                 name=f"upd_buf_{t}_{c}")
                            nc.sync.dma_start(buf[:], m_strm[c - RESIDENT])
                            nc.vector.tensor_tensor(buf[:], buf[:], tmp[:], ALU.add)
                            nc.sync.dma_start(m_strm[c - RESIDENT], buf[:])

            nc.sync.dma_start(loss_out[:], loss_acc[:])

    nc.compile()
    return nc


def prepare_inputs(x, g, M0):
    """Host-side sharding/layout prep. Returns list of per-core input maps."""
    x = np.asarray(x, dtype=np.float32)
    g = np.asarray(g, dtype=np.float32)
    M0 = np.ascontiguousarray(np.asarray(M0, dtype=np.float32))

    # i = p*64 + c  =>  view rows as [128, 64]
    M_view = M0.reshape(128, NCHUNK, P_DIM)

    h0_all = np.zeros((T_STEPS, 128, NGD), np.float32)
    p_all = np.zeros((T_STEPS, 128, NGD), np.float32)
    h0_flat = np.zeros((T_STEPS, P_DIM), np.float32)
    p_flat = np.zeros((T_STEPS, P_DIM), np.float32)
    for t in range(T_STEPS):
        q = np.tile(g[t], NXD)                    # g @ W_tile
        h0 = _f_p(q)
        p = np.outer(x[t], g[t]).reshape(P_DIM).astype(np.float32)
        h0_flat[t] = h0
        p_flat[t] = p
        h0_all[t] = h0.reshape(128, NGD)
        p_all[t] = p.reshape(128, NGD)

    import ml_dtypes
    in_maps = []
    for k in range(N_CORES):
        shard = M_view[:, :, k * J_LOC:(k + 1) * J_LOC]      # [128, 64, 1024]
        pe_part = np.ascontiguousarray(
            shard[:, :PE_CHUNKS, :]).reshape(128, PE_CHUNKS * J_LOC)
        m_hi = pe_part.astype(ml_dtypes.bfloat16)
        m_lo = (pe_part - m_hi.astype(np.float32)).astype(ml_dtypes.bfloat16)
        m_res = np.ascontiguousarray(
            shard[:, PE_CHUNKS:RESIDENT, :]).reshape(128, (RESIDENT - PE_CHUNKS) * J_LOC)
        m_strm = np.ascontiguousarray(
            shard[:, RESIDENT:, :].transpose(1, 0, 2))        # [STREAMED, 128, 1024]
        in_maps.append({
            "m_hi_in": m_hi,
            "m_lo_in": m_lo,
            "m_res_in": m_res,
            "m_strm_in": m_strm,
            "h0_sb_in": h0_all,
            "h0_loc_in": h0_flat[:, k * J_LOC:(k + 1) * J_LOC].reshape(T_STEPS, 1, J_LOC).copy(),
            "p_sb_in": p_all,
            "p_loc_in": p_flat[:, k * J_LOC:(k + 1) * J_LOC].reshape(T_STEPS, 1, J_LOC).copy(),
        })
    return in_maps


def kernel(x, g, M0):
    from concourse.bass_utils import run_bass_kernel_spmd

    in_maps = prepare_inputs(x, g, M0)
    if "nc" not in _cache:
        _cache["nc"] = build_program()
    nc = _cache["nc"]
    trace = bool(int(os.environ.get("MG_TRACE", "0")))
    res = run_bass_kernel_spmd(nc, in_maps, core_ids=list(range(N_CORES)),
                               trace=trace)
    _cache["last_result"] = res
    total = np.float32(0.0)
    for k in range(N_CORES):
        total += res.results[k]["loss_out"][0, 0]
    return np.float32(total)


# revision 13
# speedup vs baseline: 1.1667x; 1.0761x over previous
"""Trainium2 Bass kernel for nn_MemoryGame (scatter_memory).

Math (see reference):
    P = 8192, T = 4 timesteps, N_ITER = 50 attractor iterations.
    per t: h0 = f_p(tile(g_t, 128));  50x: h = f_p(kappa*h + h*(h@M))
           p = outer(x_t, g_t).ravel()
           loss_t = sum|p - h|
           M = lamda*M + yita*outer(p+h, p-h)
    output = sum_t loss_t   (scalar, fp32)

Distribution: M column-sharded over 8 cores (core k owns columns
[k*1024,(k+1)*1024)).  Each core computes its 1024-slice of a = h@M
exactly (full contraction done locally), applies the pointwise update to
its slice, and an AllGather rebuilds the full h on every core each
iteration.

Numerics: the attractor is chaotic.  Measured on this problem, matmul
operands need ~15+ mantissa bits to keep the final loss within ~1e-4;
fp16/bf16/float32r (11 bits) all land at 1e-2..2e-2.  So everything is
kept in exact fp32.  Plain fp32 PE matmul runs at 4 cycles/row, so the
GEMV is split: the PE handles PE_CHUNKS contraction chunks, and
ScalarE (per-partition-scale multiply) + VectorE (accumulate) handle
the rest into a 2-D accumulator which the PE then partition-reduces
via a ones-vector matmul accumulated into the same PSUM bank.

Memory: a 32MB fp32 shard exceeds the ~26MB usable SBUF, so RESIDENT
chunks stay SBUF-resident and the rest stream from HBM every
iteration, hidden behind the compute.

The lamda*M decay is folded into a scalar: we keep Mt = M0 +
sum_s (yita/lamda^(s+1)) u_s v_s^T and apply a = lamda^t * (h @ Mt),
saving a full read-modify-write scale pass over M per timestep.

Layout: contraction index i = p*64 + c (p = partition, c = chunk),
i.e. M's rows viewed as M0.reshape(128, 64, P); the post-AllGather
full h then loads straight into SBUF [128, 64] with no transpose, and
chunk c's stationary operand is h_sb[:, c].
"""

import os
import numpy as np

N_CORES = 8
P_DIM = 8192
NXD, NGD = 128, 64
T_STEPS = 4
N_ITER = 50
KAPPA, LAMDA, YITA = 0.8, 0.9, 0.1
NEG = 0.01

NCHUNK = 64                 # contraction chunks (128 rows each)
J_LOC = P_DIM // N_CORES    # 1024 columns per core
RESIDENT = 40               # chunks kept in SBUF
STREAMED = NCHUNK - RESIDENT
STREAM_BUFS = 4
PE_CHUNKS = 35              # chunks the PE computes (as bf16 hi/lo pairs);
                            # the rest (fp32) go to ACT+DVE

_cache = {}


def _f_p(v):
    c = np.clip(v, -1.0, 1.0)
    return np.where(c >= 0, c, NEG * c).astype(np.float32)


def build_program(debug_h=False, n_iter=None, t_run=None):
    import concourse.bacc as bacc
    import concourse.mybir as mybir
    import concourse.tile as tile

    if n_iter is None:
        n_iter = N_ITER
    if t_run is None:
        t_run = T_STEPS

    f32 = mybir.dt.float32
    AF = mybir.ActivationFunctionType
    ALU = mybir.AluOpType

    nc = bacc.Bacc(None, target_bir_lowering=False, num_devices=N_CORES)

    # register KAPPA so activation(bias=KAPPA) finds a const AP
    kap = nc.alloc_sbuf_tensor("const-kappa", [128, 1], f32)
    nc.gpsimd.memset(kap.ap(), float(KAPPA))
    nc.const_aps.aps[(f32, float(KAPPA))] = kap.ap()
    nc.all_engine_barrier()

    # ---- I/O ----
    bf16 = mybir.dt.bfloat16
    m_hi_in = nc.dram_tensor("m_hi_in", [128, PE_CHUNKS * J_LOC], bf16, kind="ExternalInput")
    m_lo_in = nc.dram_tensor("m_lo_in", [128, PE_CHUNKS * J_LOC], bf16, kind="ExternalInput")
    m_res_in = nc.dram_tensor("m_res_in", [128, (RESIDENT - PE_CHUNKS) * J_LOC], f32, kind="ExternalInput")
    m_strm_in = nc.dram_tensor("m_strm_in", [STREAMED, 128, J_LOC], f32, kind="ExternalInput")
    h0_sb_in = nc.dram_tensor("h0_sb_in", [T_STEPS, 128, NGD], f32, kind="ExternalInput")
    h0_loc_in = nc.dram_tensor("h0_loc_in", [T_STEPS, 1, J_LOC], f32, kind="ExternalInput")
    p_sb_in = nc.dram_tensor("p_sb_in", [T_STEPS, 128, NGD], f32, kind="ExternalInput")
    p_loc_in = nc.dram_tensor("p_loc_in", [T_STEPS, 1, J_LOC], f32, kind="ExternalInput")
    loss_out = nc.dram_tensor("loss_out", [1, 1], f32, kind="ExternalOutput")
    if debug_h:
        hdbg_out = nc.dram_tensor("hdbg_out", [n_iter, 1, J_LOC], f32, kind="ExternalOutput")

    # chunk engine assignment: PE gets resident chunks [0, PE_CHUNKS);
    # ACT+DVE get the rest (remaining resident + all streamed).
    dve_chunks = list(range(PE_CHUNKS, NCHUNK))

    with tile.TileContext(nc) as tc:
        with (
            tc.tile_pool(name="strm_pool", bufs=STREAM_BUFS) as strm_pool,
            tc.tile_pool(name="tmp_pool", bufs=2) as tmp_pool,
            tc.tile_pool(name="pw_pool", bufs=2) as pw_pool,
            tc.tile_pool(name="state_pool", bufs=1) as state_pool,
            tc.tile_pool(name="psum_pool", bufs=2, space="PSUM") as psum_pool,
            tc.tile_pool(name="vb_psum_pool", bufs=2, space="PSUM") as vb_psum_pool,
            tc.tile_pool(name="dram_pool", bufs=1, space="DRAM") as dram_pool,
        ):
            # ---- persistent SBUF state ----
            m_hi = state_pool.tile([128, PE_CHUNKS * J_LOC], bf16)
            m_lo = state_pool.tile([128, PE_CHUNKS * J_LOC], bf16)
            m_res = state_pool.tile([128, (RESIDENT - PE_CHUNKS) * J_LOC], f32)
            h_hi = state_pool.tile([128, NGD], bf16)
            h_lo = state_pool.tile([128, NGD], bf16)
            acc2d = state_pool.tile([128, J_LOC], f32)    # DVE-side accumulator
            h_sb = state_pool.tile([128, NGD], f32)       # full h, [p, c] = h[p*64+c]
            h_row = state_pool.tile([1, J_LOC], f32)      # local slice of h
            p_sb = state_pool.tile([128, NGD], f32)
            u_eta = state_pool.tile([128, NGD], f32)
            v_bcast = state_pool.tile([128, J_LOC], f32)
            ones_row = state_pool.tile([1, 128], f32)     # K=1 broadcast trick
            ones_col = state_pool.tile([128, 1], f32)     # partition reduction
            loss_acc = state_pool.tile([1, 1], f32)
            loss_tmp = state_pool.tile([1, 1], f32)

            # streamed part of M lives in (internal) DRAM so we can update it
            m_strm = dram_pool.tile([STREAMED, 128, J_LOC], f32)
            cc_in = dram_pool.tile([1, J_LOC], f32)

            # ---- init ----
            nc.gpsimd.memset(ones_row[:], 1.0)
            nc.gpsimd.memset(ones_col[:], 1.0)
            nc.gpsimd.memset(loss_acc[:], 0.0)
            n_ld = 8
            for big, big_in in ((m_hi, m_hi_in), (m_lo, m_lo_in)):
                step = (PE_CHUNKS * J_LOC) // n_ld
                rem = (PE_CHUNKS * J_LOC) - n_ld * step
                for i in range(n_ld):
                    e = (i + 1) * step + (rem if i == n_ld - 1 else 0)
                    nc.sync.dma_start(big[:, i * step:e], big_in[:, i * step:e])
            for i in range(RESIDENT - PE_CHUNKS):
                nc.sync.dma_start(m_res[:, i * J_LOC:(i + 1) * J_LOC],
                                  m_res_in[:, i * J_LOC:(i + 1) * J_LOC])
            for s in range(STREAMED):
                nc.sync.dma_start(m_strm[s], m_strm_in[s])

            def m_chunk(c, t, it):
                """fp32 AP for DVE-side chunk c (resident or freshly streamed)."""
                if c < RESIDENT:
                    i = c - PE_CHUNKS
                    return m_res[:, i * J_LOC:(i + 1) * J_LOC]
                tile_c = strm_pool.tile([128, J_LOC], f32, tag="mstrm",
                                        name=f"chk_{t}_{it}_{c}")
                nc.sync.dma_start(tile_c[:], m_strm[c - RESIDENT])
                return tile_c[:]

            for t in range(t_run):
                nc.sync.dma_start(h_sb[:], h0_sb_in[t])
                nc.sync.dma_start(h_row[:], h0_loc_in[t])
                nc.sync.dma_start(p_sb[:], p_sb_in[t])
                scale_t = float(LAMDA ** t)

                for it in range(n_iter):
                    with nc.named_scope(f"iter_t{t}_i{it}"):
                        acc = psum_pool.tile([1, J_LOC], f32, tag="acc",
                                             name=f"acc_{t}_{it}")

                        # --- DVE/ACT side: acc2d[p,j] = sum_c M[p,c,j]*h_sb[p,c]
                        for n, c in enumerate(dve_chunks):
                            mc = m_chunk(c, t, it)
                            hcol = h_sb[:, c:c + 1]
                            if n == 0:
                                nc.vector.tensor_scalar_mul(acc2d[:], mc, hcol)
                            else:
                                tmp = tmp_pool.tile([128, J_LOC], f32, tag="tmp",
                                                    name=f"tmp_{t}_{it}_{c}")
                                nc.scalar.activation(tmp[:], mc, AF.Copy, scale=hcol)
                                nc.vector.tensor_tensor(acc2d[:], acc2d[:], tmp[:], ALU.add)

                        # --- split h into bf16 hi/lo for the PE passes
                        nc.scalar.activation(h_hi[:], h_sb[:], AF.Copy)
                        nc.vector.tensor_tensor(h_lo[:], h_sb[:], h_hi[:], ALU.subtract)

                        # --- PE side: bf16 3-pass chunks, then += ones^T @ acc2d
                        for n in range(PE_CHUNKS):
                            sl = slice(n * J_LOC, (n + 1) * J_LOC)
                            first = n == 0
                            for pi, (hh, mm) in enumerate(
                                    ((h_hi, m_hi), (h_hi, m_lo), (h_lo, m_hi))):
                                lhsT = hh[:, n:n + 1]
                                rhs = mm[:, sl]
                                st = first and pi == 0
                                nc.tensor.matmul(acc[:, 0:512], lhsT, rhs[:, 0:512],
                                                 start=st, stop=False,
                                                 skip_group_check=True)
                                nc.tensor.matmul(acc[:, 512:1024], lhsT,
                                                 rhs[:, 512:1024],
                                                 start=st, stop=False,
                                                 skip_group_check=True)
                        nc.tensor.matmul(acc[:, 0:512], ones_col[:], acc2d[:, 0:512],
                                         start=False, stop=True, skip_group_check=True)
                        nc.tensor.matmul(acc[:, 512:1024], ones_col[:], acc2d[:, 512:1024],
                                         start=False, stop=True, skip_group_check=True)

                        # --- pointwise: h = f_p(kappa*h + h*a) with a = lamda^t*acc
                        s_t = pw_pool.tile([1, J_LOC], f32, tag="pw", name=f"s_{t}_{it}")
                        nc.scalar.activation(s_t[:], acc[:], AF.Identity,
                                             bias=float(KAPPA), scale=scale_t)
                        w_t = pw_pool.tile([1, J_LOC], f32, tag="pw", name=f"w_{t}_{it}")
                        nc.vector.tensor_tensor(w_t[:], h_row[:], s_t[:], ALU.mult)
                        c2 = pw_pool.tile([1, J_LOC], f32, tag="pw", name=f"c2_{t}_{it}")
                        nc.vector.tensor_scalar(c2[:], w_t[:], 1.0, -1.0, ALU.min, ALU.max)
                        nc.scalar.activation(h_row[:], c2[:], AF.Lrelu, alpha=float(NEG))

                        # --- exchange h slices (each AllGather needs a fresh
                        # Shared output buffer: single-writer constraint)
                        cc_out = dram_pool.tile([1, P_DIM], f32, addr_space="Shared",
                                                name=f"cc_out_{t}_{it}",
                                                tag=f"cc_out_{t}_{it}")
                        nc.sync.dma_start(cc_in[:], h_row[:])
                        nc.gpsimd.collective_compute(
                            "AllGather", ALU.bypass,
                            replica_groups=[list(range(N_CORES))],
                            ins=[cc_in[:].opt()],
                            outs=[cc_out[:].opt()],
                        )
                        cc_v = cc_out[:].rearrange("o (p c) -> (o p) c", c=NGD)
                        nc.sync.dma_start(h_sb[0:64, :], cc_v[0:64, :])
                        nc.sync.dma_start(h_sb[64:128, :], cc_v[64:128, :])
                        if debug_h:
                            nc.sync.dma_start(hdbg_out[it], h_row[:])

                # ---- timestep tail: loss, then Hebbian update if t < 3 ----
                p_loc = pw_pool.tile([1, J_LOC], f32, tag="pw", name=f"ploc_{t}")
                nc.sync.dma_start(p_loc[:], p_loc_in[t])
                v_row = pw_pool.tile([1, J_LOC], f32, tag="pw", name=f"vrow_{t}")
                nc.vector.tensor_tensor(v_row[:], p_loc[:], h_row[:], ALU.subtract)
                nc.vector.tensor_reduce(loss_tmp[:], v_row[:],
                                        mybir.AxisListType.X, ALU.add,
                                        apply_absolute_value=True)
                nc.vector.tensor_tensor(loss_acc[:], loss_acc[:], loss_tmp[:], ALU.add)

                if t < t_run - 1:
                    coef = float(YITA / (LAMDA ** (t + 1)))
                    nc.vector.tensor_tensor(u_eta[:], p_sb[:], h_sb[:], ALU.add)
                    nc.vector.tensor_scalar_mul(u_eta[:], u_eta[:], coef)
                    # v_bcast[p, j] = v_row[j] via K=1 ones matmul
                    for half in range(2):
                        vb_ps = vb_psum_pool.tile([128, 512], f32, tag="vb",
                                                  name=f"vb_{t}_{half}")
                        nc.tensor.matmul(vb_ps[:], ones_row[:],
                                         v_row[:, half * 512:(half + 1) * 512],
                                         start=True, stop=True)
                        nc.vector.tensor_copy(v_bcast[:, half * 512:(half + 1) * 512],
                                              vb_ps[:])
                    # M[:, c, :] += u_eta[:, c] * v_bcast
                    for c in range(NCHUNK):
                        tmp = tmp_pool.tile([128, J_LOC], f32, tag="tmp",
                                            name=f"upd_tmp_{t}_{c}")
                        nc.scalar.activation(tmp[:], v_bcast[:], AF.Copy,
                                             scale=u_eta[:, c:c + 1])
                        if c < PE_CHUNKS:
                            # X = hi + lo + tmp; re-split to bf16 pair
                            sl = slice(c * J_LOC, (c + 1) * J_LOC)
                            xf = tmp_pool.tile([128, J_LOC], f32, tag="tmp",
                                               name=f"upd_x_{t}_{c}")
                            nc.vector.tensor_tensor(xf[:], m_hi[:, sl], m_lo[:, sl],
                                                    ALU.add)
                            nc.vector.tensor_tensor(xf[:], xf[:], tmp[:], ALU.add)
                            nc.vector.tensor_copy(m_hi[:, sl], xf[:])
                            nc.vector.tensor_tensor(m_lo[:, sl], xf[:], m_hi[:, sl],
                                                    ALU.subtract)
                        elif c < RESIDENT:
                            i = c - PE_CHUNKS
                            mc = m_res[:, i * J_LOC:(i + 1) * J_LOC]
                            nc.vector.tensor_tensor(mc, mc, tmp[:], ALU.add)
                        else:
                            buf = strm_pool.tile([128, J_LOC], f32, tag="mstrm",
                                                 name=f"upd_buf_{t}_{c}")
                            nc.sync.dma_start(buf[:], m_strm[c - RESIDENT])
                            nc.vector.tensor_tensor(buf[:], buf[:], tmp[:], ALU.add)
                            nc.sync.dma_start(m_strm[c - RESIDENT], buf[:])

            nc.sync.dma_start(loss_out[:], loss_acc[:])

    nc.compile()
    return nc


def prepare_inputs(x, g, M0):
    """Host-side sharding/layout prep. Returns list of per-core input maps."""
    x = np.asarray(x, dtype=np.float32)
    g = np.asarray(g, dtype=np.float32)
    M0 = np.ascontiguousarray(np.asarray(M0, dtype=np.float32))

    # i = p*64 + c  =>  view rows as [128, 64]
    M_view = M0.reshape(128, NCHUNK, P_DIM)

    h0_all = np.zeros((T_STEPS, 128, NGD), np.float32)
    p_all = np.zeros((T_STEPS, 128, NGD), np.float32)
    h0_flat = np.zeros((T_STEPS, P_DIM), np.float32)
    p_flat = np.zeros((T_STEPS, P_DIM), np.float32)
    for t in range(T_STEPS):
        q = np.tile(g[t], NXD)                    # g @ W_tile
        h0 = _f_p(q)
        p = np.outer(x[t], g[t]).reshape(P_DIM).astype(np.float32)
        h0_flat[t] = h0
        p_flat[t] = p
        h0_all[t] = h0.reshape(128, NGD)
        p_all[t] = p.reshape(128, NGD)

    import ml_dtypes
    in_maps = []
    for k in range(N_CORES):
        shard = M_view[:, :, k * J_LOC:(k + 1) * J_LOC]      # [128, 64, 1024]
        pe_part = np.ascontiguousarray(
            shard[:, :PE_CHUNKS, :]).reshape(128, PE_CHUNKS * J_LOC)
        m_hi = pe_part.astype(ml_dtypes.bfloat16)
        m_lo = (pe_part - m_hi.astype(np.float32)).astype(ml_dtypes.bfloat16)
        m_res = np.ascontiguousarray(
            shard[:, PE_CHUNKS:RESIDENT, :]).reshape(128, (RESIDENT - PE_CHUNKS) * J_LOC)
        m_strm = np.ascontiguousarray(
            shard[:, RESIDENT:, :].transpose(1, 0, 2))        # [STREAMED, 128, 1024]
        in_maps.append({
            "m_hi_in": m_hi,
            "m_lo_in": m_lo,
            "m_res_in": m_res,
            "m_strm_in": m_strm,
            "h0_sb_in": h0_all,
            "h0_loc_in": h0_flat[:, k * J_LOC:(k + 1) * J_LOC].reshape(T_STEPS, 1, J_LOC).copy(),
            "p_sb_in": p_all,
            "p_loc_in": p_flat[:, k * J_LOC:(k + 1) * J_LOC].reshape(T_STEPS, 1, J_LOC).copy(),
        })
    return in_maps


def kernel(x, g, M0):
    from concourse.bass_utils import run_bass_kernel_spmd

    in_maps = prepare_inputs(x, g, M0)
    if "nc" not in _cache:
        _cache["nc"] = build_program()
    nc = _cache["nc"]
    trace = bool(int(os.environ.get("MG_TRACE", "0")))
    res = run_bass_kernel_spmd(nc, in_maps, core_ids=list(range(N_CORES)),
                               trace=trace)
    _cache["last_result"] = res
    total = np.float32(0.0)
    for k in range(N_CORES):
        total += res.results[k]["loss_out"][0, 0]
    return np.float32(total)
